# revision 31
# baseline (speedup 1.0000x reference)
"""Bahdanau additive attention kernel for Trainium2 (8 NeuronCores, SPMD).

Problem: B=32, S=2048, ENC=DEC=ATT=1024 (fp32 inputs)
  u = enc @ U_a                [B,S,A]
  w = dec @ W_a                [B,1,A]
  e = tanh(w + u) @ v_t        [B,S,1]
  align = softmax(e, axis=1)
  context = align^T @ enc      [B,1,E]
  output = tanh([dec, context] @ ffn)   [B,1,D]
  returns (output, context)

Sharding: data-parallel over batch, 4 batches per core, weights replicated.

Per-core plan (all big math in bf16, fp32 accumulation):
  - load enc batch slab with cast fp32->bf16 (SWDGE) as natural [128s, t, e] halves
  - DMA-xbar-transpose each s-tile to build encT [128e, k, s] halves
  - uT[m-chunk] = U_a_chunk.T @ encT    (PE, bf16, psum fp32)
  - tanh+bias on ACT reads psum directly, writes bf16 (bias = wT per-partition)
  - e += v_t_chunk.T @ tanh_chunk       (PE)
  - softmax: exp on ACT (accum_out gives sum), reciprocal on DVE;
    1/sum folded into the context copy-out scale
  - context = expe_cols.T @ enc_nat     (PE), scaled copy-out
  - output = tanh(catT.T @ ffn)         (PE + ACT), catT built via xbar transposes
"""

import numpy as np
import ml_dtypes

import concourse.bass as bass
import concourse.mybir as mybir
import concourse.tile as tile
from concourse import bacc
from concourse.bass_utils import run_bass_kernel_spmd

F32 = mybir.dt.float32
BF16 = mybir.dt.bfloat16
FP8 = mybir.dt.float8e4
AF = mybir.ActivationFunctionType

# u-matmul in fp8-e4m3 with DoubleRow (2x PE throughput). U_a is pre-scaled by
# 256 so its +-1/32 values sit in fp8's normal range; the tanh activation's
# scale=1/256 compensates exactly. Validated: output relmax 4.2e-3,
# context relmax 5.1e-3 vs fp32 reference.
USE_FP8 = True
U_SCALE = 256.0

B, S, E, A, D = 32, 2048, 1024, 1024, 1024
NCORES = 8
NB = B // NCORES          # 4 batches per core
P = 128
KE = E // P               # 8 contraction chunks over enc dim
MA = A // P               # 8 output chunks over att dim
KD = D // P               # 8 contraction chunks over dec dim
ST = S // P               # 16 s-tiles
SH = S // 2               # s-half size
N512 = 512


def _build_kernel_body(tc, repeat=1):
    nc = tc.nc
    enc = nc.dram_tensor("enc", [NB, S, E], F32, kind="ExternalInput")
    dec = nc.dram_tensor("dec", [NB, D], F32, kind="ExternalInput")
    U_a = nc.dram_tensor("U_a", [E, A], F32, kind="ExternalInput")
    W_a = nc.dram_tensor("W_a", [D, A], F32, kind="ExternalInput")
    v_t = nc.dram_tensor("v_t", [A, 1], F32, kind="ExternalInput")
    ffn = nc.dram_tensor("ffn", [D + E, D], F32, kind="ExternalInput")
    out = nc.dram_tensor("out", [NB, D], F32, kind="ExternalOutput")
    ctx_out = nc.dram_tensor("ctx_out", [NB, E], F32, kind="ExternalOutput")
    for _ in range(repeat):
        _build_once(tc, enc, dec, U_a, W_a, v_t, ffn, out, ctx_out)


def _build_once(tc, enc, dec, U_a, W_a, v_t, ffn, out, ctx_out):
    nc = tc.nc
    enc_r = enc.rearrange("b (t p) e -> b p t e", p=P)

    with (
        tc.tile_pool(name="weights", bufs=1) as weights,
        tc.tile_pool(name="big", bufs=1) as big,
        tc.tile_pool(name="enc_nat", bufs=3) as enc_nat_pool,
        tc.tile_pool(name="encT", bufs=2 if USE_FP8 else 3) as encT_pool,
        tc.tile_pool(name="encT8", bufs=4) as encT8_pool,
        tc.tile_pool(name="tanhp", bufs=3) as tanh_pool,
        tc.tile_pool(name="rows", bufs=1) as rows,
        tc.tile_pool(name="psum_u", bufs=2, space="PSUM") as psum_u,
        tc.tile_pool(name="psum_v", bufs=1, space="PSUM") as psum_v,
    ):
        # ---------------- weights (once per core) ----------------
        # Pool (SWDGE) is FIFO per engine: issue the loads that gate the
        # longest chains first (dec row is tiny and gates the w matmuls;
        # enc batch-0 half-0 + U_a gate the first u matmuls).
        dec16 = rows.tile([16, D], BF16, tag="pad16")
        nc.vector.memset(dec16, 0.0)
        nc.gpsimd.dma_start(out=dec16[0:NB, :], in_=dec[:, :])  # cast f32->bf16
        nat00 = enc_nat_pool.tile([P, ST // 2, E], BF16, name="nat_0_0",
                                  tag="enc_nat")
        for q in range(2):
            nc.gpsimd.dma_start(
                out=nat00[:, q * 4 : (q + 1) * 4, :],
                in_=enc_r[0, :, q * 4 : q * 4 + 4, :],
            )
        U_sb = weights.tile([P, KE, A], BF16)
        U_r = U_a.rearrange("(k p) a -> p k a", p=P)
        nc.gpsimd.dma_start(out=U_sb[:, 0 : KE // 2, :], in_=U_r[:, 0 : KE // 2, :])
        nc.gpsimd.dma_start(out=U_sb[:, KE // 2 :, :], in_=U_r[:, KE // 2 :, :])
        if USE_FP8:
            # U8[p, kk, j, a] = U_a[(kk*2+j)*128+p, a] * 256 (same element order
            # as U_sb); split per kk so the first u matmuls start early
            U8 = weights.tile([P, KE // 2, 2, A], FP8)
            for kk in range(KE // 2):
                nc.vector.tensor_scalar_mul(
                    U8[:, kk, :, :].rearrange("p j a -> p (j a)"),
                    U_sb[:, 2 * kk : 2 * kk + 2, :].rearrange("p k a -> p (k a)"),
                    U_SCALE,
                )
        W_sb = big.tile([P, KD, A], BF16, tag="big")
        W_r = W_a.rearrange("(k p) a -> p k a", p=P)
        nc.gpsimd.dma_start(out=W_sb[:, :, 0:512], in_=W_r[:, :, 0:512])
        nc.gpsimd.dma_start(out=W_sb[:, :, 512:1024], in_=W_r[:, :, 512:1024])
        nat01 = enc_nat_pool.tile([P, ST // 2, E], BF16, name="nat_0_1",
                                  tag="enc_nat")
        for q in range(2):
            nc.gpsimd.dma_start(
                out=nat01[:, q * 4 : (q + 1) * 4, :],
                in_=enc_r[0, :, 8 + q * 4 : 8 + q * 4 + 4, :],
            )
        v_sb = weights.tile([P, MA], BF16)
        nc.gpsimd.dma_start(out=v_sb, in_=v_t.rearrange("(m p) one -> p (m one)", p=P))

        # catT holds [decT | contextT] chunks: catT[p, c, j] = cat[j, c*128+p]
        catT = weights.tile([P, 2 * KE, 16], BF16)
        nc.sync.dma_start(out=catT[:, 0:KE, :], in_=dec16, transpose=True)

        # wT[p, m, b] = w[b, m*128+p]
        wT_ps = psum_v.tile([P, MA, NB], F32, tag="vec")
        for m in range(MA):
            for k in range(KD):
                nc.tensor.matmul(
                    wT_ps[:, m, :],
                    lhsT=W_sb[:, k, m * P : (m + 1) * P],
                    rhs=catT[:, k, 0:NB],
                    start=(k == 0),
                    stop=(k == KD - 1),
                )
        wT = weights.tile([P, MA, NB], F32)
        nc.scalar.copy(wT[:, 0 : MA // 2, :], wT_ps[:, 0 : MA // 2, :])
        nc.scalar.copy(wT[:, MA // 2 :, :], wT_ps[:, MA // 2 :, :])

        # ffn reuses W_sb's slot once W_a is consumed
        ffn_sb = big.tile([P, 2 * KE, D], BF16, tag="big")
        nc.gpsimd.dma_start(out=ffn_sb, in_=ffn.rearrange("(k p) d -> p k d", p=P))

        # context rows (bf16, padded to 16 partitions) for the final transpose
        ctx16 = rows.tile([16, E], BF16, tag="pad16")
        nc.vector.memset(ctx16, 0.0)

        # ---------------- per-batch pipeline ----------------
        for b in range(NB):
            # load enc batch (cast to bf16), natural layout, two s-halves.
            # batch 0 halves were issued above ahead of the other weights.
            enc_nat = []
            for h in range(2):
                if b == 0:
                    enc_nat.append(nat00 if h == 0 else nat01)
                    continue
                nat = enc_nat_pool.tile(
                    [P, ST // 2, E], BF16, name=f"nat_{b}_{h}", tag="enc_nat"
                )
                for q in range(2):
                    t0 = h * 8 + q * 4
                    nc.gpsimd.dma_start(
                        out=nat[:, q * 4 : (q + 1) * 4, :],
                        in_=enc_r[b, :, t0 : t0 + 4, :],
                    )
                enc_nat.append(nat)

            # transpose to encT s-halves: eT[p, k, s'] = enc[b, h*1024+s', k*128+p]
            encT = []
            encT8 = []
            for h in range(2):
                if USE_FP8:
                    # one xbar call per quarter (4 s-tiles): in [128, 4096]
                    # contiguous -> stage [128, 32, 128] where c = t*8 + k;
                    # DVE cast reshuffles to encT8 [p, (kk j), s] right behind
                    eT8 = encT8_pool.tile(
                        [P, KE // 2, 2, SH], FP8, name=f"encT8_{b}_{h}", tag="encT8"
                    )
                    eT8v = eT8.rearrange("p kk j s -> p (kk j) s")
                    for q in range(2):
                        stage = encT_pool.tile(
                            [P, 4 * KE, P], BF16, name=f"stage_{b}_{h}_{q}",
                            tag="encT",
                        )
                        nc.sync.dma_start(
                            out=stage,
                            in_=enc_nat[h][:, q * 4 : (q + 1) * 4, :],
                            transpose=True,
                        )
                        # stage[p, t*8+k, j] -> eT8v[p, k, q*512 + t*128 + j]
                        nc.vector.tensor_copy(
                            eT8v[:, :, q * 512 : (q + 1) * 512].rearrange(
                                "p k (t j) -> p k t j", t=4
                            ),
                            stage.rearrange("p (t k) j -> p k t j", t=4),
                        )
                    encT8.append(eT8)
                else:
                    eT = encT_pool.tile(
                        [P, KE, SH], BF16, name=f"encT_{b}_{h}", tag="encT"
                    )
                    for t in range(8):
                        nc.sync.dma_start(
                            out=eT[:, :, t * P : (t + 1) * P],
                            in_=enc_nat[h][:, t, :],
                            transpose=True,
                        )
                    encT.append(eT)

            # u (uT layout), fused tanh+bias, e accumulation.
            # h is the OUTER loop: the h=0 block (~28us of PE work) hides the
            # h=1 half's load+transpose chain.
            e_ps = psum_v.tile([1, S], F32, name=f"e_ps_{b}", tag="vec")
            expe = rows.tile([1, S], BF16, name="expe", tag="expe")
            expe16 = rows.tile([ST, P], BF16, name="expe16", tag="expe16")
            esums = []
            for h in range(2):
                for m in range(MA):
                    u_ps = psum_u.tile([P, SH], F32, name="u_ps", tag="u")
                    if USE_FP8:
                        for n in range(2):
                            for kk in range(KE // 2):
                                nc.tensor.matmul(
                                    u_ps[:, n * N512 : (n + 1) * N512],
                                    lhsT=U8[:, kk, :, m * P : (m + 1) * P],
                                    rhs=encT8[h][:, kk, :, n * N512 : (n + 1) * N512],
                                    start=(kk == 0),
                                    stop=(kk == KE // 2 - 1),
                                    perf_mode=mybir.MatmulPerfMode.DoubleRow,
                                )
                    else:
                        for n in range(2):
                            for k in range(KE):
                                nc.tensor.matmul(
                                    u_ps[:, n * N512 : (n + 1) * N512],
                                    lhsT=U_sb[:, k, m * P : (m + 1) * P],
                                    rhs=encT[h][:, k, n * N512 : (n + 1) * N512],
                                    start=(k == 0),
                                    stop=(k == KE - 1),
                                )
                    th = tanh_pool.tile([P, SH], BF16, name="th", tag="th")
                    nc.scalar.activation(
                        th, u_ps, AF.Tanh,
                        bias=wT[:, m, b : b + 1],
                        scale=(1.0 / U_SCALE) if USE_FP8 else 1.0,
                    )
                    for n in range(2):
                        nc.tensor.matmul(
                            e_ps[:, h * SH + n * N512 : h * SH + (n + 1) * N512],
                            lhsT=v_sb[:, m : m + 1],
                            rhs=th[:, n * N512 : (n + 1) * N512],
                            start=(m == 0),
                            stop=(m == MA - 1),
                        )
                # softmax pieces per half so the h=0 exp/reshape overlaps the
                # h=1 u-block (e is bounded, skip max-subtraction)
                esum_h = rows.tile([1, 1], F32, name=f"esum{h}", tag=f"esum{h}")
                nc.scalar.activation(
                    expe[:, h * SH : (h + 1) * SH],
                    e_ps[:, h * SH : (h + 1) * SH],
                    AF.Exp,
                    accum_out=esum_h,
                )
                esums.append(esum_h)
                nc.gpsimd.dma_start(
                    out=expe16[h * 8 : (h + 1) * 8, :],
                    in_=expe[:, h * SH : (h + 1) * SH].rearrange(
                        "one (t p) -> one t p", t=8
                    ),
                )
            esum = rows.tile([1, 1], F32, name="esum", tag="esum")
            nc.vector.tensor_add(esum, esums[0], esums[1])
            rsum = rows.tile([1, 1], F32, name="rsum", tag="rsum")
            nc.vector.reciprocal(rsum, esum)
            expe_cols = rows.tile([P, ST], BF16, name="expe_cols", tag="expe_cols")
            nc.sync.dma_start(out=expe_cols, in_=expe16, transpose=True)

            # context (unnormalized), 1/sum folded into copy-out scale
            ctx_ps = psum_v.tile([1, E], F32, name=f"ctx_ps_{b}", tag="vec")
            for t in range(ST):
                for n in range(2):
                    nc.tensor.matmul(
                        ctx_ps[:, n * N512 : (n + 1) * N512],
                        lhsT=expe_cols[:, t : t + 1],
                        rhs=enc_nat[t // 8][:, t % 8, n * N512 : (n + 1) * N512],
                        start=(t == 0),
                        stop=(t == ST - 1),
                    )
            ctx_row = rows.tile([1, E], F32, name="ctx_row", tag="ctx_row")
            nc.scalar.activation(ctx_row, ctx_ps, AF.Copy, scale=rsum)
            nc.sync.dma_start(out=ctx_out[b : b + 1, :], in_=ctx_row)
            nc.gpsimd.dma_start(out=ctx16[b : b + 1, :], in_=ctx_row)  # cast

        # ---------------- final ffn (all batches at once) ----------------
        nc.sync.dma_start(out=catT[:, KE : 2 * KE, :], in_=ctx16, transpose=True)
        out_ps = psum_v.tile([NB, D], F32, tag="vec")
        for c in range(2 * KE):
            for n in range(2):
                nc.tensor.matmul(
                    out_ps[:, n * N512 : (n + 1) * N512],
                    lhsT=catT[:, c, 0:NB],
                    rhs=ffn_sb[:, c, n * N512 : (n + 1) * N512],
                    start=(c == 0),
                    stop=(c == 2 * KE - 1),
                )
        out_sb = weights.tile([NB, D], F32)
        nc.scalar.activation(out_sb, out_ps, AF.Tanh)
        nc.sync.dma_start(out=out[:, :], in_=out_sb)


_NC_CACHE = None


def _get_nc(repeat=1):
    global _NC_CACHE
    if repeat != 1:
        nc = bacc.Bacc(None, target_bir_lowering=False)
        with tile.TileContext(nc) as tc:
            _build_kernel_body(tc, repeat=repeat)
        nc.compile()
        return nc
    if _NC_CACHE is None:
        nc = bacc.Bacc(None, target_bir_lowering=False)
        with tile.TileContext(nc) as tc:
            _build_kernel_body(tc)
        nc.compile()
        _NC_CACHE = nc
    return _NC_CACHE


def kernel(encoder_hidden_states, decoder_hidden_state, U_a, W_a, v_t, ffn,
           _trace=False):
    enc = np.ascontiguousarray(np.asarray(encoder_hidden_states, dtype=np.float32))
    dec = np.ascontiguousarray(
        np.asarray(decoder_hidden_state, dtype=np.float32).reshape(B, D)
    )
    U = np.ascontiguousarray(np.asarray(U_a, dtype=np.float32))
    W = np.ascontiguousarray(np.asarray(W_a, dtype=np.float32))
    v = np.ascontiguousarray(np.asarray(v_t, dtype=np.float32))
    F = np.ascontiguousarray(np.asarray(ffn, dtype=np.float32))

    nc = _get_nc()
    in_maps = []
    for c in range(NCORES):
        sl = slice(c * NB, (c + 1) * NB)
        in_maps.append(
            {
                "enc": enc[sl],
                "dec": dec[sl],
                "U_a": U,
                "W_a": W,
                "v_t": v,
                "ffn": F,
            }
        )
    res = run_bass_kernel_spmd(nc, in_maps, core_ids=list(range(NCORES)),
                               trace=_trace)

    output = np.empty((B, 1, D), dtype=np.float32)
    context = np.empty((B, 1, E), dtype=np.float32)
    for c in range(NCORES):
        sl = slice(c * NB, (c + 1) * NB)
        output[sl, 0, :] = res.results[c]["out"]
        context[sl, 0, :] = res.results[c]["ctx_out"]
    if _trace:
        return (output, context), res
    return (output, context)


if __name__ == "__main__":
    import reference

    inputs = {k: np.asarray(v) for k, v in reference.setup_inputs().items()}
    (o, c) = kernel(**inputs)
    print("output", o.shape, o.dtype, "context", c.shape, c.dtype)
